# revision 8
# baseline (speedup 1.0000x reference)
"""Trainium2 Bass kernel for the NoisyTopK MoE layer (B=2,T=2048,D=1024,H=4096,O=1024,E=8,K=2).

Strategy (expert-parallel, 8 cores = 8 experts):
  * Host: compute the full noisy-top2 routing (indices AND softmax gates,
    tiny numpy), gather each expert's tokens, pad to a common capacity C
    (multiple of 64).
  * Device (per core, SPMD — same program, per-expert data):
      expert FFN: out = (relu(x @ W1 + b1) @ W2 + b2) * gate
      fused MM1->MM2 per H-slice, f16 matmuls, W2 resident in SBUF,
      W1 streamed per chunk of 384 tokens, per-token gate scalar from host.
  * Host: scatter-add the per-expert outputs back to [B,T,O]
    (equivalent to the all-reduce of the gated combine).
"""

import os
import time

import numpy as np

P = 128
B, T, D, H, O, E = 2, 2048, 1024, 4096, 1024, 8
KD = D // P   # 8  k-tiles over D
KH = H // P   # 32 k-tiles over H (= number of m-slices of MM1)
NM = H // P   # 32 m-slices
OS = 2        # O-slices of 512
TB_PER_CHUNK = 3  # 384 tokens per chunk

_NC_CACHE = {}
LAST_RUN = {}


def _build_nc(C):
    import concourse.mybir as mybir
    import concourse.tile as tile
    from concourse import bacc

    f32 = mybir.dt.float32
    f16 = mybir.dt.float16
    AF = mybir.ActivationFunctionType

    assert C % 64 == 0
    NTB = (C + P - 1) // P  # token blocks; last may be 64 wide
    blocks = [P] * (C // P) + ([C % P] if C % P else [])
    chunks = []  # (first block idx, global token offset, [block sizes])
    b0 = 0
    while b0 < NTB:
        n = min(TB_PER_CHUNK, NTB - b0)
        chunks.append((b0, sum(blocks[:b0]), blocks[b0 : b0 + n]))
        b0 += n

    # Bacc (not plain Bass): its compile() pass splits multi-wait matmuls
    # (HW allows a single sync-wait on the fused LDWEIGHTS+MATMULT).
    nc = bacc.Bacc()
    CH0 = min(TB_PER_CHUNK * P, C)
    xh_d = nc.declare_dram_parameter("xh", [P, KD, C], f16, isOutput=False)
    # chunk-0 x, contiguous per partition: 1 DMA segment per partition row
    # instead of KD, so the startup-critical load is packet-cheap
    xh0_d = nc.declare_dram_parameter("xh0", [P, KD * CH0], f16, isOutput=False)
    w1_d = nc.declare_dram_parameter("w1s", [NM, P, KD, P], f16, isOutput=False)
    w2_d = nc.declare_dram_parameter("w2s", [P, KH, O], f16, isOutput=False)
    b1_d = nc.declare_dram_parameter("b1s", [P, NM], f32, isOutput=False)
    b2_d = nc.declare_dram_parameter("b2e", [1, O], f32, isOutput=False)
    g_d = nc.declare_dram_parameter("gates", [P, NTB], f32, isOutput=False)
    out_d = nc.declare_dram_parameter("out", [C, O], f32, isOutput=True)

    with tile.TileContext(nc) as tc:
        with (
            tc.tile_pool(name="singles", bufs=1) as singles,
            tc.tile_pool(name="w1pool", bufs=3) as w1pool,
            tc.tile_pool(name="xpool", bufs=2 * KD) as xpool,
            tc.tile_pool(name="hpool", bufs=8) as hpool,
            tc.tile_pool(name="spool", bufs=2) as spool,
            tc.tile_pool(name="psA", bufs=6, space="PSUM") as psA,
            tc.tile_pool(name="psB", bufs=2, space="PSUM") as psB,
        ):
            # ---- resident tensors ----
            # W2 stays resident in SBUF; its slices are loaded just-in-time
            # inside chunk 0's m-loop so the first matmuls don't stall on a
            # bulk 8MB load.
            w2_sb = singles.tile([P, KH, O], f16)
            b1_sb = singles.tile([P, NM], f32)
            nc.sync.dma_start(b1_sb[:], b1_d[:])
            b2_sb = singles.tile([P, O], f32)
            g_sb = singles.tile([P, NTB], f32)

            def emit_setup_small():
                # evict-phase constants — deferred so they don't sit ahead
                # of the chunk-0 x/W1 loads in the DMA queues
                nc.sync.dma_start(g_sb[:], g_d[:])
                nc.sync.dma_start(b2_sb[:], b2_d[0].partition_broadcast(P))

            # MM2 trails MM1 by DELTA H-slices: the PE always has independent
            # MM1 work while MM2 waits on relu eviction / psum-slot release.
            DELTA = 6

            def emit_x_load(ci):
                # x for one chunk, one tile per ko: tile-granular deps let
                # MM1 ko=0 start as soon as its 96KB slice lands instead of
                # waiting for the whole 0.77MB chunk
                b0c, t0c, bsz = chunks[ci]
                nt = sum(bsz)
                xs = [
                    xpool.tile(
                        [P, TB_PER_CHUNK * P], f16, tag="xs", name=f"xs{ko}"
                    )
                    for ko in range(KD)
                ]
                x0v = xh0_d[:].rearrange("p (k t) -> p k t", k=KD)
                for ko in range(KD):
                    src = x0v[:, ko] if ci == 0 else xh_d[:, ko, t0c : t0c + nt]
                    nc.sync.dma_start(xs[ko][:, :nt], src)
                return xs

            xs_next = emit_x_load(0)
            for ci, (b0c, t0c, bsz) in enumerate(chunks):
                nt = sum(bsz)
                ntb = len(bsz)
                bofs = [sum(bsz[:j]) for j in range(ntb)]
                xs = xs_next
                accs = [
                    [
                        psA.tile([P, 512], f32, tag="acc", name=f"acc_{j}_{osl}")
                        for osl in range(OS)
                    ]
                    for j in range(ntb)
                ]
                # pad64: a 64-wide final block would give MM2 a 64-col
                # stationary (disables FWL, +50ns/MM measured); zero-pad hm
                # so its MM2s run as full 128-col stationary instead
                pad64 = bsz[-1] == 64
                hms = {}
                for m in range(NM):
                    if ci == 0 and m == 4:
                        # deferred past the first MM1s so the DMA queues
                        # drain the critical-path loads first
                        emit_setup_small()
                    if m == 18 and ci + 1 < len(chunks):
                        # prefetch next chunk's x while this chunk's m-loop
                        # keeps the PE saturated
                        xs_next = emit_x_load(ci + 1)
                    if m < NM:
                        w1t = w1pool.tile([P, KD, P], f16, tag="w1t")
                        nc.sync.dma_start(w1t[:], w1_d[m])
                        if ci == 0:
                            nc.sync.dma_start(w2_sb[:, m, :], w2_d[:, m, :])
                        hps = psB.tile([P, TB_PER_CHUNK * P], f32, tag="mm1ps")
                        hw = hps[:, :nt]
                        for ko in range(KD):
                            nc.tensor.matmul(
                                hw,
                                w1t[:, ko, :],
                                xs[ko][:, :nt],
                                start=(ko == 0),
                                stop=(ko == KD - 1),
                            )
                        hm = hpool.tile([P, TB_PER_CHUNK * P], f16, tag="hm")
                        nc.scalar.activation(
                            hm[:, :nt], hw, AF.Relu, bias=b1_sb[:, m : m + 1]
                        )
                        if pad64:
                            nc.vector.memset(hm[:, nt : nt + 64], 0.0)
                        hms[m] = hm
                    if m >= DELTA:
                        mm = m - DELTA
                        hm2 = hms.pop(mm)
                        for j in range(ntb):
                            bs = 128 if (pad64 and bsz[j] == 64) else bsz[j]
                            for osl in range(OS):
                                nc.tensor.matmul(
                                    accs[j][osl][:bs],
                                    hm2[:, bofs[j] : bofs[j] + bs],
                                    w2_sb[:, mm, osl * 512 : (osl + 1) * 512],
                                    start=(mm == 0),
                                    stop=(mm == NM - 1),
                                )

                # ---- pipeline drain, block-major: finish block j's
                # accumulation, then evict it while block j+1 drains ----
                for j in range(ntb):
                    bs = bsz[j]
                    bsm = 128 if (pad64 and bs == 64) else bs
                    for mm in range(NM - DELTA, NM):
                        hm2 = hms[mm]
                        for osl in range(OS):
                            nc.tensor.matmul(
                                accs[j][osl][:bsm],
                                hm2[:, bofs[j] : bofs[j] + bsm],
                                w2_sb[:, mm, osl * 512 : (osl + 1) * 512],
                                start=(mm == 0),
                                stop=(mm == NM - 1),
                            )
                    # evict: (acc + b2) * gate -> DRAM
                    st = spool.tile([P, O], f32, tag="st")
                    for osl in range(OS):
                        sl = slice(osl * 512, (osl + 1) * 512)
                        nc.vector.tensor_add(
                            st[:bs, sl], accs[j][osl][:bs], b2_sb[:bs, sl]
                        )
                        nc.vector.tensor_scalar_mul(
                            st[:bs, sl],
                            st[:bs, sl],
                            g_sb[:bs, b0c + j : b0c + j + 1],
                        )
                    g0 = t0c + bofs[j]
                    nc.sync.dma_start(out_d[g0 : g0 + bs, :], st[:bs, :])
                hms.clear()

    nc.finalize()
    return nc


def _routing_host(xf, nf, Wg, bg, Wn, bn):
    """Top-2 expert mask AND the sparse softmax gates per token."""
    logits = xf @ Wg + bg
    nl = xf @ Wn + bn
    sp = np.logaddexp(0.0, nl)
    noisy = logits + nf * sp
    order = np.argpartition(-noisy, 2, axis=1)[:, :2]
    mask = np.zeros(noisy.shape, dtype=bool)
    mask[np.arange(noisy.shape[0])[:, None], order] = True
    # softmax over the two selected logits (matches reference: softmax of
    # the -inf-masked logits, then L1-normalize — a numeric no-op)
    neg = np.where(mask, noisy, -np.inf)
    mx = neg.max(axis=1, keepdims=True)
    ex = np.exp(neg - mx)
    gates = ex / ex.sum(axis=1, keepdims=True)
    gates[~mask] = 0.0
    return mask, gates.astype(np.float32)


def _prep_core(xf, gates, idx, C, W1e, b1e, W2e, b2e, e):
    n = len(idx)
    x_g = np.zeros((C, D), np.float32)
    x_g[:n] = xf[idx]
    NTB = (C + P - 1) // P
    g_g = np.zeros((NTB * P,), np.float32)
    g_g[:n] = gates[idx, e]
    xh = np.ascontiguousarray(
        x_g.reshape(C, KD, P).transpose(2, 1, 0)
    ).astype(np.float16)
    CH0 = min(TB_PER_CHUNK * P, C)
    return {
        "xh": xh,
        "xh0": np.ascontiguousarray(xh[:, :, :CH0]).reshape(P, KD * CH0),
        "w1s": np.ascontiguousarray(
            W1e.reshape(KD, P, NM, P).transpose(2, 1, 0, 3)
        ).astype(np.float16),
        "w2s": np.ascontiguousarray(
            W2e.reshape(KH, P, O).transpose(1, 0, 2)
        ).astype(np.float16),
        "b1s": np.ascontiguousarray(b1e.reshape(NM, P).T),
        "b2e": b2e[None, :].astype(np.float32),
        "gates": np.ascontiguousarray(g_g.reshape(NTB, P).T),
    }


def kernel(x, noise, Wg, bg, Wn, bn, W1, b1, W2, b2):
    from concourse.bass_utils import run_bass_kernel_spmd

    x = np.asarray(x, np.float32)
    noise = np.asarray(noise, np.float32)
    Wg = np.asarray(Wg, np.float32)
    bg = np.asarray(bg, np.float32)
    Wn = np.asarray(Wn, np.float32)
    bn = np.asarray(bn, np.float32)
    W1 = np.asarray(W1, np.float32)
    b1 = np.asarray(b1, np.float32)
    W2 = np.asarray(W2, np.float32)
    b2 = np.asarray(b2, np.float32)

    Bx, Tx, _ = x.shape
    ntok = Bx * Tx
    xf = x.reshape(ntok, D)
    nf = noise.reshape(ntok, E)

    mask, gates = _routing_host(xf, nf, Wg, bg, Wn, bn)
    idx = [np.nonzero(mask[:, e])[0] for e in range(E)]
    C = max(P, int(np.ceil(max(len(i) for i in idx) / 64) * 64))

    if C not in _NC_CACHE:
        _NC_CACHE[C] = _build_nc(C)
    nc = _NC_CACHE[C]

    in_maps = [
        _prep_core(xf, gates, idx[e], C, W1[e], b1[e], W2[e], b2[e], e)
        for e in range(E)
    ]

    trace = bool(os.environ.get("MOE_TRACE"))
    t0 = time.time()
    res = run_bass_kernel_spmd(
        nc, in_maps, list(range(E)), trace=trace
    )
    t1 = time.time()
    LAST_RUN.clear()
    LAST_RUN.update(
        wall_s=t1 - t0,
        exec_time_ns=res.exec_time_ns,
        trace=res.instructions_and_trace[1] if res.instructions_and_trace else None,
    )

    out = np.zeros((ntok, O), np.float32)
    for e in range(E):
        n = len(idx[e])
        y = res.results[e]["out"].reshape(C, O)
        out[idx[e]] += y[:n]
    return out.reshape(Bx, Tx, O)
